# revision 31
# baseline (speedup 1.0000x reference)
"""Trainium2 Bass kernel for nn_Aggregation (SAN-style position-dependent
3x3 depthwise aggregation with share_planes=8).

  out[n, c, h, w] = sum_k input[n, c, h+dh(k), w+dw(k)] * weight[n, c//8, k, h*W+w]

Sharding: data-parallel over batch N=8 across the 8 NeuronCores (one image
per core, no collectives).

Per-core design (input [256,56,56], weight [32,9,3136] per image):
  - SBUF partition p = q*32 + g: q in 0..3 = 14-row quarter of the image,
    g in 0..31 = weight group. The 8 share-channels of a group live in the
    free dimension, so each weight element is read via a stride-0 broadcast
    AP instead of being replicated.
  - The host pre-packs ONE flat fp16 slab per partition:
      [ x chunk0 (s=0..3, 4x900) | weight (9x784) | identity row (128) |
        x chunk1 (s=4..7, 4x900) ]
    where each 900-elem x block is a zero-padded flat image quarter
    (guard + 16 rows [14 + 2 halo] * 56 cols + guard + pad), so each tap
    (dh, dw) is a single contiguous 784-slice at offset 1 + (dh+1)*56 + dw.
    Column wrap-around reads are neutralized by zeroing the weight's edge
    columns host-side (those taps multiply out-of-image zero padding in the
    exact computation).  Packing gives the DMA maximal contiguous runs per
    partition.  Each DMA-issuing engine owns one serial queue stream with a
    multi-us fixed startup, and consumers wait on whole-DMA completion, so
    the input is split by consumption stage across three queues: x chunk 0
    (the compute gate) on sync-HWDGE, identity + weight planes in three
    consumption-ordered groups on scalar-HWDGE, x chunk 1 on the
    late-starting gpsimd-SWDGE queue.
  - fp16 storage: DVE tensor_tensor runs in 2x perf mode (needs 16-bit,
    step 1, 4B-aligned APs -> a second, one-element-shifted copy of the x
    slab, built on-chip by ScalarE, gives every tap an even base offset).
    ALU math is fp32 internally.
  - Compute: VectorE does only the 9 tap multiplies per chunk (2x mode);
    TensorE accumulates the 9 product arrays into PSUM with identity-
    stationary matmuls (fp32 accumulation, start/stop has_written groups),
    concurrently on its own SBUF ports; ScalarE drains PSUM -> SBUF with an
    fp16 downcast.  Work is split into 2 share-axis chunks (PSUM = 3
    two-bank pair tiles + 1 single per chunk; matmuls target bank-aligned
    448-col slices, drains read whole pairs in one strided AP).
  - Schedule (from trace analysis): the 3 DMA queues (sync/scalar HWDGE,
    gpsimd SWDGE) arm staggered ~8.4/9.6/10.3us after a ~7us framework
    preamble and share the ~358GB/s HBM ceiling, so pieces are ordered by
    consumption time per ring with x1 strictly last; the first two
    multiplies are gated only by [x0 s0:2 | w0 | w1] and taps 0-2 run as
    s01/s23 half-multiplies so the DVE starts while x0 s2:4 is in flight.
    The DVE then streams all multiplies gaplessly (~31.5us, the hard
    floor: 7.23M products/core at 2 fp16/cycle/lane); tiny warm-up ops
    absorb the first-instruction dispatch cost on DVE/ACT.  The final
    multiply is split so closing matmuls, pair drains (VectorE + ScalarE
    in parallel) and per-pair output DMAs on both idle HWDGE rings
    pipeline behind it.
"""

import numpy as np

N, C, H, W = 8, 256, 56, 56
G, KK, L = 32, 9, 3136          # weight groups, taps, spatial
SHARE = 8                        # C // G
Q = 4                            # row-quarters
RQ = H // Q                      # 14 rows per quarter
LQ = RQ * W                      # 784 pixels per quarter
XA = 900                         # guard + 16*56 + guard + pad (even)

DTYPE = "float16"                # on-chip storage dtype
SPLIT = 2                        # share-axis chunks (overlap DMA/compute)
SC = SHARE // SPLIT              # share-channels per chunk
# tap order: xa-based taps (dw=+-1) first so compute can start before the
# on-chip xb shift-copies finish; xb-based taps (dw=0) last.
TAP_ORDER = [0, 2, 3, 5, 6, 8, 1, 4, 7]
MM = 448                         # matmul free-dim tile (7 * 448 = 3136)

# packed input slab column offsets (fp16 elements per partition).
# weight planes are stored in TAP_ORDER so they stream off HBM in exactly
# the order compute consumes them (region-based deps let tap j start as
# soon as its plane lands).  Layout is ordered by consumption time; the
# head ladder is: x0 shares 0-1 + plane 0 land first (first multiplies run
# on the s01 half), shares 2-3 arrive in parallel on the gpsimd queue.
OFF_X0 = 0
OFF_W0 = SC * XA                 # 3600: plane 0
OFF_W1 = OFF_W0 + LQ             # 4384: plane 1
OFF_ID = OFF_W1 + LQ             # 5168
OFF_W2 = OFF_ID + 128            # 5296: planes 2-8
OFF_X1 = OFF_W2 + 7 * LQ         # 10784
SLAB = OFF_X1 + SC * XA          # 14384

_CACHE = {}


def _build():
    import concourse.bacc as bacc
    import concourse.mybir as mybir
    import concourse.tile as tile

    dt = getattr(mybir.dt, DTYPE)

    nc = bacc.Bacc("TRN2", target_bir_lowering=False, debug=False)
    inp = nc.dram_tensor("inp", [128, SLAB], dt, kind="ExternalInput")
    out = nc.dram_tensor("out", [128, SHARE, LQ], dt, kind="ExternalOutput")

    with tile.TileContext(nc) as tc:
        with (
            tc.tile_pool(name="main", bufs=1) as pool,
            tc.tile_pool(name="prod", bufs=4) as ppool,
            tc.tile_pool(name="psum", bufs=1, space="PSUM") as psum_pool,
        ):
            inbuf = pool.tile([128, SLAB], dt)
            xb = pool.tile([128, SHARE, XA - 4], dt)

            # warm the DVE/ACT dispatch paths while waiting on the first
            # DMAs — the first op on a cold engine pays ~0.8us of i-cache /
            # uop-table setup that would otherwise land on the critical path.
            scr = pool.tile([128, 16], dt)
            nc.vector.memset(scr[:, 0:8], 0.0)
            nc.vector.tensor_mul(scr[:, 8:16], scr[:, 0:8], scr[:, 0:8])
            nc.scalar.copy(scr[:, 8:16], scr[:, 0:8])

            # DMA queue plan.  The queue streams behave near-serially at the
            # head (~285GB/s effective aggregate) and arm staggered (sync
            # ~8.4us, scalar ~9.6, gpsimd-SWDGE ~10.6-11.7 first packet);
            # back-to-back DMAs on a ring pipeline with a ~0.6-1us desc-gen
            # gap after short transfers; sems fire ~0.9us after last byte.
            # Ladder: x0 s0-1 (sync) + plane 0 (scalar) land first and the
            # first three taps run on the s01 half; s2-3 rides the
            # late-arming gpsimd queue in parallel and the s23 halves catch
            # up before tap 3; remaining planes alternate sync/scalar as
            # singles in consumption order; x1 follows on gpsimd.
            # x1 rides LAST on the HWDGE rings (ring FIFO defers it until
            # the weight ladder has drained) — putting it on gpsimd next to
            # s23 starves the weight queues of HBM bandwidth exactly when
            # planes 1-4 are due.
            for eng, a, b in (
                (nc.sync, OFF_X0, 1800),                            # x0 s0:2
                (nc.scalar, OFF_W0, OFF_W1),                        # w0
                (nc.gpsimd, OFF_W1, OFF_W2),                        # w1 + ident
                (nc.sync, 1800, OFF_W0),                            # x0 s2:4
                (nc.scalar, OFF_W2, OFF_W2 + LQ),                   # w2
                (nc.scalar, OFF_W2 + LQ, OFF_W2 + 3 * LQ),          # w3-4
                (nc.sync, OFF_W2 + 3 * LQ, OFF_W2 + 5 * LQ),        # w5-6
                (nc.scalar, OFF_W2 + 5 * LQ, OFF_X1),               # w7-8
                (nc.sync, OFF_X1, OFF_X1 + 1800),                   # x1 s4:6
                (nc.scalar, OFF_X1 + 1800, SLAB),                   # x1 s6:8
            ):
                eng.dma_start(out=inbuf[:, a:b], in_=inp.ap()[:, a:b])

            xa_views = [
                inbuf[:, OFF_X0 : OFF_X0 + SC * XA].rearrange(
                    "p (s l) -> p s l", s=SC
                ),
                inbuf[:, OFF_X1 : OFF_X1 + SC * XA].rearrange(
                    "p (s l) -> p s l", s=SC
                ),
            ]
            wt01 = inbuf[:, OFF_W0 : OFF_W0 + 2 * LQ].rearrange(
                "p (k l) -> p k l", k=2
            )
            wt2 = inbuf[:, OFF_W2 : OFF_W2 + 7 * LQ].rearrange(
                "p (k l) -> p k l", k=7
            )

            def w_plane(j):
                if j < 2:
                    return wt01[:, j : j + 1, :]
                return wt2[:, j - 2 : j - 1, :]

            ident = inbuf[:, OFF_ID : OFF_ID + 128]

            # xb = x shifted by one element: gives dw=0 taps an even base;
            # built on-chip by the otherwise-idle ScalarE.
            for c in range(SPLIT):
                nc.scalar.copy(
                    xb[:, c * SC : (c + 1) * SC, :], xa_views[c][:, :, 1 : XA - 3]
                )

            def x_ap_for(c, k):
                dh, dw = k // 3 - 1, k % 3 - 1
                if dw == 0:
                    base = (dh + 1) * W      # even; xb = xa shifted by 1
                    return xb[:, c * SC : (c + 1) * SC, base : base + LQ]
                base = 1 + (dh + 1) * W + dw  # even by construction
                return xa_views[c][:, :, base : base + LQ]

            outbuf = pool.tile([128, SHARE, LQ], dt)
            ofl = out.ap().rearrange("p s l -> p (s l)")
            nhalf = (SC * LQ) // MM          # matmul tiles per chunk
            for c in range(SPLIT):
                s0, s1 = c * SC, (c + 1) * SC
                # PSUM as three 2-bank pair tiles + one single: matmuls
                # target 448-col slices at bank-aligned offsets, drains read
                # whole pairs in one strided AP (fewer, larger copies on the
                # critical tail).
                pairs = [
                    psum_pool.tile(
                        [128, 1024], mybir.dt.float32,
                        name=f"bankp{c}_{p}", tag=f"bankp{p}",
                    )
                    for p in range(3)
                ] + [
                    psum_pool.tile(
                        [128, MM], mybir.dt.float32,
                        name=f"bankp{c}_3", tag=f"bankp3",
                    )
                ]

                def bank_ap(t):
                    if t < 6:
                        o = (t % 2) * 512
                        return pairs[t // 2][:, o : o + MM]
                    return pairs[3][:]

                def drain(eng, t0, t1, ob):
                    """copy banks [t0:t1) (pair-aligned) to ob columns."""
                    for p in range(t0 // 2, (t1 + 1) // 2):
                        if 2 * p + 1 < t1 and p < 3:
                            src = pairs[p][:].rearrange(
                                "q (b x) -> q b x", b=2
                            )[:, :, 0:MM]
                            dst = ob[:, 2 * p * MM : (2 * p + 2) * MM].rearrange(
                                "q (b x) -> q b x", b=2
                            )
                        else:
                            src = pairs[3][:]
                            dst = ob[:, 6 * MM : 7 * MM]
                        if eng is nc.vector:
                            nc.vector.tensor_copy(out=dst, in_=src)
                        else:
                            nc.scalar.copy(dst, src)
                def emit_half(j, prod, lo, hi, t0, t1, stop, eng=None):
                    """multiply shares [lo:hi) of tap j into prod, then run
                    the closing matmuls for banks [t0:t1)."""
                    prod_s = prod[:].rearrange("p (s l) -> p s l", s=SC)
                    w_ap = w_plane(j).broadcast_to([128, SC, LQ])
                    x_ap = x_ap_for(c, TAP_ORDER[j])
                    (eng or nc.vector).tensor_mul(
                        prod_s[:, lo:hi, :], x_ap[:, lo:hi, :], w_ap[:, lo:hi, :]
                    )
                    for t in range(t0, t1):
                        nc.tensor.matmul(
                            bank_ap(t), ident,
                            prod[:, t * MM : (t + 1) * MM],
                            start=(j == 0), stop=stop, skip_group_check=True,
                        )

                if c == 0:
                    # head ladder: x0 arrives as s01 pair, then s2, then s3
                    # (sync-ring singles) — taps 0-2 run as a matching
                    # s01/s2/s3 multiply ladder so the DVE starts ~2.7us
                    # before x0 fully lands.  Straddling banks wait on both
                    # neighbouring pieces via region deps.
                    head = [
                        ppool.tile([128, SC * LQ], dt, name=f"hprod{i}")
                        for i in range(3)
                    ]
                    for j in range(3):
                        emit_half(j, head[j], 0, 2, 0, 3, False)
                    for j in range(3):
                        emit_half(j, head[j], 2, SC, 3, nhalf, False)
                    rest = range(3, KK)
                    for j in rest:
                        prod = ppool.tile([128, SC * LQ], dt)
                        emit_half(j, prod, 0, SC, 0, nhalf, j == KK - 1)
                else:
                    # chunk 1: all planes are resident — merge same-row tap
                    # pairs (x windows a constant stride apart: dw -1/+1 are
                    # 2 cols apart in the xa slab, dh rows 56 apart in xb)
                    # into single 4D-AP multiplies, halving DVE op count.
                    APc = None
                    for pj, (j0, delta) in enumerate(
                        [(0, 2), (2, 2), (4, 2), (6, 56)]
                    ):
                        prod = pool.tile(
                            [128, 2 * SC * LQ], dt, name=f"mprod{pj % 2}"
                        )
                        prod4 = prod[:].rearrange(
                            "p (j s l) -> p j s l", j=2, s=SC
                        )
                        if j0 == 0:
                            wv = wt01[:, 0:2, :]
                        else:
                            wv = wt2[:, j0 - 2 : j0, :]
                        w4 = wv.unsqueeze(2).broadcast_to([128, 2, SC, LQ])
                        x3 = x_ap_for(c, TAP_ORDER[j0])
                        if APc is None:
                            APc = type(x3)
                        x4 = APc(
                            x3.tensor, x3.offset,
                            [x3.ap[0], [delta, 2], x3.ap[1], x3.ap[2]],
                        )
                        nc.vector.tensor_mul(prod4, x4, w4)
                        for jl in range(2):
                            for t in range(nhalf):
                                nc.tensor.matmul(
                                    bank_ap(t), ident,
                                    prod[:, jl * SC * LQ + t * MM :
                                         jl * SC * LQ + (t + 1) * MM],
                                    start=(j0 + jl == 0), stop=False,
                                    skip_group_check=True,
                                )
                    # final tap split at a share boundary so the first banks'
                    # closing matmuls (and the drain+output pipeline behind
                    # them) start ~1.6us earlier.
                    prod = ppool.tile([128, SC * LQ], dt)
                    emit_half(KK - 1, prod, 0, 2, 0, 3, True)
                    emit_half(KK - 1, prod, 2, SC, 3, nhalf, True)
                ob = outbuf[:, s0:s1, :].rearrange("p s l -> p (s l)")
                col0 = s0 * LQ
                if c == SPLIT - 1:
                    # tail: VectorE is idle after its last mult — split the
                    # PSUM drain between VectorE (bank pairs 0-1) and ScalarE
                    # (pair 2 + single), and stream each piece on the idle
                    # queues as soon as it lands in SBUF.
                    for p in range(2):
                        drain(nc.vector, 2 * p, 2 * p + 2, ob)
                        nc.sync.dma_start(
                            out=ofl[:, col0 + 2 * p * MM : col0 + (2 * p + 2) * MM],
                            in_=ob[:, 2 * p * MM : (2 * p + 2) * MM],
                        )
                    drain(nc.scalar, 4, 6, ob)
                    nc.scalar.dma_start(
                        out=ofl[:, col0 + 4 * MM : col0 + 6 * MM],
                        in_=ob[:, 4 * MM : 6 * MM],
                    )
                    drain(nc.scalar, 6, nhalf, ob)
                    nc.scalar.dma_start(
                        out=ofl[:, col0 + 6 * MM : col0 + nhalf * MM],
                        in_=ob[:, 6 * MM : nhalf * MM],
                    )
                else:
                    # chunk 0: drains on ScalarE; output streams in 2-bank
                    # pieces on the sync queue (idle, fully hidden under
                    # chunk-1 compute).
                    for p in range(3):
                        drain(nc.scalar, 2 * p, 2 * p + 2, ob)
                        nc.sync.dma_start(
                            out=ofl[:, col0 + 2 * p * MM : col0 + (2 * p + 2) * MM],
                            in_=ob[:, 2 * p * MM : (2 * p + 2) * MM],
                        )
                    drain(nc.scalar, 6, nhalf, ob)
                    nc.sync.dma_start(
                        out=ofl[:, col0 + (nhalf - 1) * MM : col0 + nhalf * MM],
                        in_=ob[:, (nhalf - 1) * MM : nhalf * MM],
                    )

    nc.compile()
    return nc


def _get_nc():
    if "nc" not in _CACHE:
        _CACHE["nc"] = _build()
    return _CACHE["nc"]


def _prep_shards(input, weight):
    np_dt = np.dtype(DTYPE)
    # padded image per (g, s): rows -1..56 zero-padded
    inp = np.asarray(input).reshape(N, G, SHARE, H, W)
    pad = np.zeros((N, G, SHARE, H + 2, W), dtype=np_dt)
    pad[:, :, :, 1 : H + 1, :] = inp
    # x slab: [N, q, g, s, XA]
    xh = np.zeros((N, Q, G, SHARE, XA), dtype=np_dt)
    for q in range(Q):
        xh[:, q, :, :, 1 : 1 + 16 * W] = pad[:, :, :, q * RQ : q * RQ + 16, :].reshape(
            N, G, SHARE, 16 * W
        )
    xh = xh.reshape(N, 128, SHARE, XA)

    # weight: [N, (q g), k, LQ] with out-of-image edge columns zeroed
    wh = np.asarray(weight).astype(np_dt).reshape(N, G, KK, H, W)
    for k in range(KK):
        dwk = k % 3 - 1
        if dwk == -1:
            wh[:, :, k, :, 0] = 0
        elif dwk == 1:
            wh[:, :, k, :, W - 1] = 0
    wh = (
        wh.reshape(N, G, KK, Q, LQ)
        .transpose(0, 3, 1, 2, 4)
        .reshape(N, 128, KK * LQ)
    )

    wh = wh.reshape(N, 128, KK, LQ)[:, :, TAP_ORDER, :]   # consumption order

    slab = np.empty((N, 128, SLAB), dtype=np_dt)
    slab[:, :, OFF_X0:OFF_W0] = xh[:, :, :SC, :].reshape(N, 128, SC * XA)
    slab[:, :, OFF_W0:OFF_ID] = wh[:, :, :2, :].reshape(N, 128, 2 * LQ)
    slab[:, :, OFF_ID:OFF_W2] = np.eye(128, dtype=np_dt)[None]
    slab[:, :, OFF_W2:OFF_X1] = wh[:, :, 2:, :].reshape(N, 128, 7 * LQ)
    slab[:, :, OFF_X1:SLAB] = xh[:, :, SC:, :].reshape(N, 128, SC * XA)
    return [{"inp": np.ascontiguousarray(slab[n])} for n in range(N)]


def _unpack_out(res_list):
    # res: [128, SHARE, LQ] per core -> (N, C, H, W) float32
    o = np.stack([r["out"] for r in res_list], axis=0).astype(np.float32)
    o = o.reshape(N, Q, G, SHARE, LQ).transpose(0, 2, 3, 1, 4)
    return np.ascontiguousarray(o.reshape(N, C, H, W))


def kernel(input, weight):
    from concourse.bass_utils import run_bass_kernel_spmd

    nc = _get_nc()
    in_maps = _prep_shards(input, weight)
    res = run_bass_kernel_spmd(nc, in_maps, core_ids=list(range(N)))
    return _unpack_out(res.results)



# revision 32
# speedup vs baseline: 1.0237x; 1.0237x over previous
"""Trainium2 Bass kernel for nn_Aggregation (SAN-style position-dependent
3x3 depthwise aggregation with share_planes=8).

  out[n, c, h, w] = sum_k input[n, c, h+dh(k), w+dw(k)] * weight[n, c//8, k, h*W+w]

Sharding: data-parallel over batch N=8 across the 8 NeuronCores (one image
per core, no collectives).

Per-core design (input [256,56,56], weight [32,9,3136] per image):
  - SBUF partition p = q*32 + g: q in 0..3 = 14-row quarter of the image,
    g in 0..31 = weight group. The 8 share-channels of a group live in the
    free dimension, so each weight element is read via a stride-0 broadcast
    AP instead of being replicated.
  - The host pre-packs ONE flat fp16 slab per partition:
      [ x chunk0 (s=0..3, 4x900) | weight (9x784) | identity row (128) |
        x chunk1 (s=4..7, 4x900) ]
    where each 900-elem x block is a zero-padded flat image quarter
    (guard + 16 rows [14 + 2 halo] * 56 cols + guard + pad), so each tap
    (dh, dw) is a single contiguous 784-slice at offset 1 + (dh+1)*56 + dw.
    Column wrap-around reads are neutralized by zeroing the weight's edge
    columns host-side (those taps multiply out-of-image zero padding in the
    exact computation).  Packing gives the DMA maximal contiguous runs per
    partition.  Each DMA-issuing engine owns one serial queue stream with a
    multi-us fixed startup, and consumers wait on whole-DMA completion, so
    the input is split by consumption stage across three queues: x chunk 0
    (the compute gate) on sync-HWDGE, identity + weight planes in three
    consumption-ordered groups on scalar-HWDGE, x chunk 1 on the
    late-starting gpsimd-SWDGE queue.
  - fp16 storage: DVE tensor_tensor runs in 2x perf mode (needs 16-bit,
    step 1, 4B-aligned APs -> a second, one-element-shifted copy of the x
    slab, built on-chip by ScalarE, gives every tap an even base offset).
    ALU math is fp32 internally.
  - Compute: VectorE does only the 9 tap multiplies per chunk (2x mode);
    TensorE accumulates the 9 product arrays into PSUM with identity-
    stationary matmuls (fp32 accumulation, start/stop has_written groups),
    concurrently on its own SBUF ports; ScalarE drains PSUM -> SBUF with an
    fp16 downcast.  Work is split into 2 share-axis chunks (PSUM = 3
    two-bank pair tiles + 1 single per chunk; matmuls target bank-aligned
    448-col slices, drains read whole pairs in one strided AP).
  - Schedule (from trace analysis): the 3 DMA queues (sync/scalar HWDGE,
    gpsimd SWDGE) arm staggered ~8.4/9.6/10.3us after a ~7us framework
    preamble and share the ~358GB/s HBM ceiling, so pieces are ordered by
    consumption time per ring with x1 strictly last; the first two
    multiplies are gated only by [x0 s0:2 | w0 | w1] and taps 0-2 run as
    s01/s23 half-multiplies so the DVE starts while x0 s2:4 is in flight.
    The DVE then streams all multiplies gaplessly (~31.5us, the hard
    floor: 7.23M products/core at 2 fp16/cycle/lane); tiny warm-up ops
    absorb the first-instruction dispatch cost on DVE/ACT.  The final
    multiply is split so closing matmuls, pair drains (VectorE + ScalarE
    in parallel) and per-pair output DMAs on both idle HWDGE rings
    pipeline behind it.
"""

import numpy as np

N, C, H, W = 8, 256, 56, 56
G, KK, L = 32, 9, 3136          # weight groups, taps, spatial
SHARE = 8                        # C // G
Q = 4                            # row-quarters
RQ = H // Q                      # 14 rows per quarter
LQ = RQ * W                      # 784 pixels per quarter
XA = 900                         # guard + 16*56 + guard + pad (even)

DTYPE = "float16"                # on-chip storage dtype
SPLIT = 2                        # share-axis chunks (overlap DMA/compute)
SC = SHARE // SPLIT              # share-channels per chunk
# tap order: xa-based taps (dw=+-1) first so compute can start before the
# on-chip xb shift-copies finish; xb-based taps (dw=0) last.
TAP_ORDER = [0, 2, 3, 5, 6, 8, 1, 4, 7]
MM = 448                         # matmul free-dim tile (7 * 448 = 3136)

# packed input slab column offsets (fp16 elements per partition).
# weight planes are stored in TAP_ORDER so they stream off HBM in exactly
# the order compute consumes them (region-based deps let tap j start as
# soon as its plane lands).  Layout is ordered by consumption time; the
# head ladder is: x0 shares 0-1 + plane 0 land first (first multiplies run
# on the s01 half), shares 2-3 arrive in parallel on the gpsimd queue.
OFF_X0 = 0
OFF_W0 = SC * XA                 # 3600: plane 0
OFF_W1 = OFF_W0 + LQ             # 4384: plane 1
OFF_ID = OFF_W1 + LQ             # 5168
OFF_W2 = OFF_ID + 128            # 5296: planes 2-8
OFF_X1 = OFF_W2 + 7 * LQ         # 10784
SLAB = OFF_X1 + SC * XA          # 14384

_CACHE = {}


def _build():
    import concourse.bacc as bacc
    import concourse.mybir as mybir
    import concourse.tile as tile

    dt = getattr(mybir.dt, DTYPE)

    nc = bacc.Bacc("TRN2", target_bir_lowering=False, debug=False)
    inp = nc.dram_tensor("inp", [128, SLAB], dt, kind="ExternalInput")
    out = nc.dram_tensor("out", [128, SHARE, LQ], dt, kind="ExternalOutput")

    with tile.TileContext(nc) as tc:
        with (
            tc.tile_pool(name="main", bufs=1) as pool,
            tc.tile_pool(name="prod", bufs=4) as ppool,
            tc.tile_pool(name="psum", bufs=1, space="PSUM") as psum_pool,
        ):
            inbuf = pool.tile([128, SLAB], dt)
            xb = pool.tile([128, SHARE, XA - 4], dt)

            # warm the DVE/ACT dispatch paths while waiting on the first
            # DMAs — the first op on a cold engine pays ~0.8us of i-cache /
            # uop-table setup that would otherwise land on the critical path.
            scr = pool.tile([128, 16], dt)
            nc.vector.memset(scr[:, 0:8], 0.0)
            nc.vector.tensor_mul(scr[:, 8:16], scr[:, 0:8], scr[:, 0:8])
            nc.scalar.copy(scr[:, 8:16], scr[:, 0:8])

            # DMA queue plan.  The queue streams behave near-serially at the
            # head (~285GB/s effective aggregate) and arm staggered (sync
            # ~8.4us, scalar ~9.6, gpsimd-SWDGE ~10.6-11.7 first packet);
            # back-to-back DMAs on a ring pipeline with a ~0.6-1us desc-gen
            # gap after short transfers; sems fire ~0.9us after last byte.
            # Ladder: x0 s0-1 (sync) + plane 0 (scalar) land first and the
            # first three taps run on the s01 half; s2-3 rides the
            # late-arming gpsimd queue in parallel and the s23 halves catch
            # up before tap 3; remaining planes alternate sync/scalar as
            # singles in consumption order; x1 follows on gpsimd.
            # x1 rides LAST on the HWDGE rings (ring FIFO defers it until
            # the weight ladder has drained) — putting it on gpsimd next to
            # s23 starves the weight queues of HBM bandwidth exactly when
            # planes 1-4 are due.
            for eng, a, b in (
                (nc.sync, OFF_X0, 1800),                            # x0 s0:2
                (nc.scalar, OFF_W0, OFF_W1),                        # w0
                (nc.gpsimd, OFF_W1, OFF_W2),                        # w1 + ident
                (nc.sync, 1800, OFF_W0),                            # x0 s2:4
                (nc.scalar, OFF_W2, OFF_W2 + LQ),                   # w2
                (nc.scalar, OFF_W2 + LQ, OFF_W2 + 3 * LQ),          # w3-4
                (nc.sync, OFF_W2 + 3 * LQ, OFF_W2 + 5 * LQ),        # w5-6
                (nc.scalar, OFF_W2 + 5 * LQ, OFF_X1),               # w7-8
                (nc.sync, OFF_X1, OFF_X1 + 1800),                   # x1 s4:6
                (nc.scalar, OFF_X1 + 1800, SLAB),                   # x1 s6:8
            ):
                eng.dma_start(out=inbuf[:, a:b], in_=inp.ap()[:, a:b])

            xa_views = [
                inbuf[:, OFF_X0 : OFF_X0 + SC * XA].rearrange(
                    "p (s l) -> p s l", s=SC
                ),
                inbuf[:, OFF_X1 : OFF_X1 + SC * XA].rearrange(
                    "p (s l) -> p s l", s=SC
                ),
            ]
            wt01 = inbuf[:, OFF_W0 : OFF_W0 + 2 * LQ].rearrange(
                "p (k l) -> p k l", k=2
            )
            wt2 = inbuf[:, OFF_W2 : OFF_W2 + 7 * LQ].rearrange(
                "p (k l) -> p k l", k=7
            )

            def w_plane(j):
                if j < 2:
                    return wt01[:, j : j + 1, :]
                return wt2[:, j - 2 : j - 1, :]

            ident = inbuf[:, OFF_ID : OFF_ID + 128]

            # xb = x shifted by one element: gives dw=0 taps an even base;
            # built on-chip by the otherwise-idle ScalarE.
            for c in range(SPLIT):
                nc.scalar.copy(
                    xb[:, c * SC : (c + 1) * SC, :], xa_views[c][:, :, 1 : XA - 3]
                )

            def x_ap_for(c, k):
                dh, dw = k // 3 - 1, k % 3 - 1
                if dw == 0:
                    base = (dh + 1) * W      # even; xb = xa shifted by 1
                    return xb[:, c * SC : (c + 1) * SC, base : base + LQ]
                base = 1 + (dh + 1) * W + dw  # even by construction
                return xa_views[c][:, :, base : base + LQ]

            outbuf = pool.tile([128, SHARE, LQ], dt)
            ofl = out.ap().rearrange("p s l -> p (s l)")
            nhalf = (SC * LQ) // MM          # matmul tiles per chunk
            for c in range(SPLIT):
                s0, s1 = c * SC, (c + 1) * SC
                # PSUM as three 2-bank pair tiles + one single: matmuls
                # target 448-col slices at bank-aligned offsets, drains read
                # whole pairs in one strided AP (fewer, larger copies on the
                # critical tail).
                pairs = [
                    psum_pool.tile(
                        [128, 1024], mybir.dt.float32,
                        name=f"bankp{c}_{p}", tag=f"bankp{p}",
                    )
                    for p in range(3)
                ] + [
                    psum_pool.tile(
                        [128, MM], mybir.dt.float32,
                        name=f"bankp{c}_3", tag=f"bankp3",
                    )
                ]

                def bank_ap(t):
                    if t < 6:
                        o = (t % 2) * 512
                        return pairs[t // 2][:, o : o + MM]
                    return pairs[3][:]

                def drain(eng, t0, t1, ob):
                    """copy banks [t0:t1) (pair-aligned) to ob columns."""
                    for p in range(t0 // 2, (t1 + 1) // 2):
                        if 2 * p + 1 < t1 and p < 3:
                            src = pairs[p][:].rearrange(
                                "q (b x) -> q b x", b=2
                            )[:, :, 0:MM]
                            dst = ob[:, 2 * p * MM : (2 * p + 2) * MM].rearrange(
                                "q (b x) -> q b x", b=2
                            )
                        else:
                            src = pairs[3][:]
                            dst = ob[:, 6 * MM : 7 * MM]
                        if eng is nc.vector:
                            nc.vector.tensor_copy(out=dst, in_=src)
                        else:
                            nc.scalar.copy(dst, src)
                def emit_half(j, prod, lo, hi, t0, t1, stop, eng=None):
                    """multiply shares [lo:hi) of tap j into prod, then run
                    the closing matmuls for banks [t0:t1)."""
                    prod_s = prod[:].rearrange("p (s l) -> p s l", s=SC)
                    w_ap = w_plane(j).broadcast_to([128, SC, LQ])
                    x_ap = x_ap_for(c, TAP_ORDER[j])
                    (eng or nc.vector).tensor_mul(
                        prod_s[:, lo:hi, :], x_ap[:, lo:hi, :], w_ap[:, lo:hi, :]
                    )
                    for t in range(t0, t1):
                        nc.tensor.matmul(
                            bank_ap(t), ident,
                            prod[:, t * MM : (t + 1) * MM],
                            start=(j == 0), stop=stop, skip_group_check=True,
                        )

                if c == 0:
                    # head ladder: x0 arrives as s01 pair, then s2, then s3
                    # (sync-ring singles) — taps 0-2 run as a matching
                    # s01/s2/s3 multiply ladder so the DVE starts ~2.7us
                    # before x0 fully lands.  Straddling banks wait on both
                    # neighbouring pieces via region deps.
                    head = [
                        ppool.tile([128, SC * LQ], dt, name=f"hprod{i}")
                        for i in range(3)
                    ]
                    for j in range(3):
                        emit_half(j, head[j], 0, 2, 0, 3, False)
                    for j in range(3):
                        emit_half(j, head[j], 2, SC, 3, nhalf, False)
                    rest = range(3, KK)
                else:
                    rest = range(KK)
                for j in rest:
                    prod = ppool.tile([128, SC * LQ], dt)
                    if c == SPLIT - 1 and j == KK - 1:
                        # split the final multiply at a share boundary so the
                        # first banks' closing matmuls (and the drain+output
                        # pipeline behind them) start ~1.6us earlier.
                        emit_half(j, prod, 0, 2, 0, 3, True)
                        emit_half(j, prod, 2, SC, 3, nhalf, True)
                    else:
                        emit_half(j, prod, 0, SC, 0, nhalf, j == KK - 1)
                ob = outbuf[:, s0:s1, :].rearrange("p s l -> p (s l)")
                col0 = s0 * LQ
                if c == SPLIT - 1:
                    # tail: VectorE is idle after its last mult — split the
                    # PSUM drain between VectorE (bank pairs 0-1) and ScalarE
                    # (pair 2 + single), and stream each piece on the idle
                    # queues as soon as it lands in SBUF.
                    for p in range(2):
                        drain(nc.vector, 2 * p, 2 * p + 2, ob)
                        nc.sync.dma_start(
                            out=ofl[:, col0 + 2 * p * MM : col0 + (2 * p + 2) * MM],
                            in_=ob[:, 2 * p * MM : (2 * p + 2) * MM],
                        )
                    drain(nc.scalar, 4, 6, ob)
                    nc.scalar.dma_start(
                        out=ofl[:, col0 + 4 * MM : col0 + 6 * MM],
                        in_=ob[:, 4 * MM : 6 * MM],
                    )
                    drain(nc.scalar, 6, nhalf, ob)
                    nc.scalar.dma_start(
                        out=ofl[:, col0 + 6 * MM : col0 + nhalf * MM],
                        in_=ob[:, 6 * MM : nhalf * MM],
                    )
                else:
                    # chunk 0: drains on ScalarE; output streams in 2-bank
                    # pieces on the sync queue (idle, fully hidden under
                    # chunk-1 compute).
                    for p in range(3):
                        drain(nc.scalar, 2 * p, 2 * p + 2, ob)
                        nc.sync.dma_start(
                            out=ofl[:, col0 + 2 * p * MM : col0 + (2 * p + 2) * MM],
                            in_=ob[:, 2 * p * MM : (2 * p + 2) * MM],
                        )
                    drain(nc.scalar, 6, nhalf, ob)
                    nc.sync.dma_start(
                        out=ofl[:, col0 + (nhalf - 1) * MM : col0 + nhalf * MM],
                        in_=ob[:, (nhalf - 1) * MM : nhalf * MM],
                    )

    nc.compile()
    return nc


def _get_nc():
    if "nc" not in _CACHE:
        _CACHE["nc"] = _build()
    return _CACHE["nc"]


def _prep_shards(input, weight):
    np_dt = np.dtype(DTYPE)
    # padded image per (g, s): rows -1..56 zero-padded
    inp = np.asarray(input).reshape(N, G, SHARE, H, W)
    pad = np.zeros((N, G, SHARE, H + 2, W), dtype=np_dt)
    pad[:, :, :, 1 : H + 1, :] = inp
    # x slab: [N, q, g, s, XA]
    xh = np.zeros((N, Q, G, SHARE, XA), dtype=np_dt)
    for q in range(Q):
        xh[:, q, :, :, 1 : 1 + 16 * W] = pad[:, :, :, q * RQ : q * RQ + 16, :].reshape(
            N, G, SHARE, 16 * W
        )
    xh = xh.reshape(N, 128, SHARE, XA)

    # weight: [N, (q g), k, LQ] with out-of-image edge columns zeroed
    wh = np.asarray(weight).astype(np_dt).reshape(N, G, KK, H, W)
    for k in range(KK):
        dwk = k % 3 - 1
        if dwk == -1:
            wh[:, :, k, :, 0] = 0
        elif dwk == 1:
            wh[:, :, k, :, W - 1] = 0
    wh = (
        wh.reshape(N, G, KK, Q, LQ)
        .transpose(0, 3, 1, 2, 4)
        .reshape(N, 128, KK * LQ)
    )

    wh = wh.reshape(N, 128, KK, LQ)[:, :, TAP_ORDER, :]   # consumption order

    slab = np.empty((N, 128, SLAB), dtype=np_dt)
    slab[:, :, OFF_X0:OFF_W0] = xh[:, :, :SC, :].reshape(N, 128, SC * XA)
    slab[:, :, OFF_W0:OFF_ID] = wh[:, :, :2, :].reshape(N, 128, 2 * LQ)
    slab[:, :, OFF_ID:OFF_W2] = np.eye(128, dtype=np_dt)[None]
    slab[:, :, OFF_W2:OFF_X1] = wh[:, :, 2:, :].reshape(N, 128, 7 * LQ)
    slab[:, :, OFF_X1:SLAB] = xh[:, :, SC:, :].reshape(N, 128, SC * XA)
    return [{"inp": np.ascontiguousarray(slab[n])} for n in range(N)]


def _unpack_out(res_list):
    # res: [128, SHARE, LQ] per core -> (N, C, H, W) float32
    o = np.stack([r["out"] for r in res_list], axis=0).astype(np.float32)
    o = o.reshape(N, Q, G, SHARE, LQ).transpose(0, 2, 3, 1, 4)
    return np.ascontiguousarray(o.reshape(N, C, H, W))


def kernel(input, weight):
    from concourse.bass_utils import run_bass_kernel_spmd

    nc = _get_nc()
    in_maps = _prep_shards(input, weight)
    res = run_bass_kernel_spmd(nc, in_maps, core_ids=list(range(N)))
    return _unpack_out(res.results)

